# revision 1
# baseline (speedup 1.0000x reference)
"""Masked multi-head attention on 8 TRN2 NeuronCores.

Sharding: 8 cores = 2 batches x 4 head-groups (4 heads of 64 dims each).
Each core computes full causal attention for its (batch, 4-head) slice:
  Q^T/K^T projections (dh on partitions) with biases folded in as K=1 matmuls,
  V kept in k-major layout augmented with a ones column (the PV matmul then
  yields numerator rows 0-63 and the softmax denominator in row 64 of one PSUM
  accumulation), scores S^T = K^T-tile.T @ Q^T per 128-key tile with
  above-diagonal tiles skipped and diagonal tiles column-restricted and
  additively masked pre-exp, exp on ScalarE (no max subtraction: scores are
  ~N(0,1) so exp cannot overflow), denominator broadcast via a K=1 matmul,
  fast reciprocal and multiply. Matmul operands are bf16 (fp32 PSUM
  accumulation). Output is attn^T per core; the host transposes and
  concatenates.
"""
import threading
from contextlib import ExitStack

import ml_dtypes
import numpy as np

import concourse.bass as bass
import concourse.tile as tile
from concourse import bacc, mybir
from concourse.bass_utils import run_bass_kernel_spmd

F32 = mybir.dt.float32
MMDT = mybir.dt.bfloat16
NPDT = ml_dtypes.bfloat16

B, T, C = 2, 2048, 1024
H, DH = 16, 64
HPC = 4            # heads per core
RPC = HPC * DH     # 256 output channels per core
NCT = C // 128     # 8 contraction tiles
NQC = T // 512     # 4 query chunks
NKT = T // 128     # 16 key tiles
NEG = -1.0e30


def _build(n_iter: int = 1, parts: str = "all"):
    nc = bacc.Bacc("TRN2", target_bir_lowering=False, debug=False)
    xt = nc.dram_tensor("xt", [C, T], MMDT, kind="ExternalInput").ap()
    wq = nc.dram_tensor("wq", [C, RPC], MMDT, kind="ExternalInput").ap()
    wk = nc.dram_tensor("wk", [C, RPC], MMDT, kind="ExternalInput").ap()
    wv = nc.dram_tensor("wv", [C, RPC], MMDT, kind="ExternalInput").ap()
    bq = nc.dram_tensor("bq", [128, 2], F32, kind="ExternalInput").ap()
    bk = nc.dram_tensor("bk", [128, 2], F32, kind="ExternalInput").ap()
    bv = nc.dram_tensor("bv", [128, 2], F32, kind="ExternalInput").ap()
    mask = nc.dram_tensor("mask", [128, 128], F32, kind="ExternalInput").ap()
    ones = nc.dram_tensor("ones", [1, 512], MMDT, kind="ExternalInput").ap()
    ot = nc.dram_tensor("ot", [RPC, T], F32, kind="ExternalOutput").ap()

    do_proj = parts in ("proj", "scores", "all")
    do_attn = parts in ("scores", "all")
    do_pv = parts == "all"

    with tile.TileContext(nc) as tc, ExitStack() as ctx:
        if n_iter > 1:
            ctx.enter_context(tc.For_i(0, n_iter))
        per = ctx.enter_context(tc.tile_pool(name="per", bufs=1))
        wrk = ctx.enter_context(tc.tile_pool(name="wrk", bufs=4))
        tl = ctx.enter_context(tc.tile_pool(name="tl", bufs=2))
        ps = ctx.enter_context(tc.tile_pool(name="ps", bufs=1, space="PSUM"))

        # ---- load phase (small tensors first, then xt per c-tile) ----
        wq_s = per.tile([128, NCT, RPC], MMDT, tag="wq")
        wk_s = per.tile([128, NCT, RPC], MMDT, tag="wk")
        wv_s = per.tile([128, NCT, RPC], MMDT, tag="wv")
        nc.sync.dma_start(wq_s[:], wq.rearrange("(c p) m -> p c m", p=128))
        nc.sync.dma_start(wk_s[:], wk.rearrange("(c p) m -> p c m", p=128))
        nc.sync.dma_start(wv_s[:], wv.rearrange("(c p) m -> p c m", p=128))
        bq_s = per.tile([128, 2], F32, tag="bq")
        bk_s = per.tile([128, 2], F32, tag="bk")
        bv_s = per.tile([128, 2], F32, tag="bv")
        nc.sync.dma_start(bq_s[:], bq[:])
        nc.sync.dma_start(bk_s[:], bk[:])
        nc.sync.dma_start(bv_s[:], bv[:])
        mask_s = per.tile([128, 128], F32, tag="mask")
        nc.sync.dma_start(mask_s[:], mask[:])
        ones_s = per.tile([1, 512], MMDT, tag="ones")
        nc.sync.dma_start(ones_s[:], ones[:])

        xt_s = [per.tile([128, T], MMDT, tag=f"xt{ct}", name=f"xt_s{ct}")
                for ct in range(NCT)]
        for ct in range(NCT):
            nc.sync.dma_start(xt_s[ct][:], xt[128 * ct:128 * (ct + 1), :])

        # V augmented with a ones column: [k-part, ktile, head, 65]
        # 128-elem head stride: xbar-transpose dst must be 256B-aligned
        v_aug = per.tile([128, NKT, HPC, 2 * DH], MMDT, tag="vaug")
        nc.gpsimd.memset(v_aug[:, :, :, DH:2 * DH], 1.0)

        if parts == "load":
            dump = tl.tile([128, 64], F32, tag="dump")
            nc.vector.tensor_copy(dump[:, 0:16], xt_s[7][:, 0:16])
            nc.vector.tensor_copy(dump[:, 16:32], wq_s[:, 0, 0:16])
            nc.vector.tensor_copy(dump[:, 32:48], wk_s[:, 0, 0:16])
            nc.vector.tensor_copy(dump[:, 48:64], wv_s[:, 0, 0:16])
            nc.sync.dma_start(ot[0:128, 0:64], dump[:])

        # ---- projections ----
        qt_s = per.tile([128, 2, T], MMDT, tag="qt")
        kt_s = per.tile([128, 2, T], MMDT, tag="kt")
        vt_s = per.tile([128, 2, T], MMDT, tag="vt")

        def emit_proj_group(w_s, b_s, o_s, gr, chk, tag):
            """Generator: one projection PSUM group, step-by-step."""
            pq = ps.tile([128, 512], F32, tag=tag, bufs=(2 if tag == "s2" else 1),
                         name=f"pq_{o_s.tensor.name}_{gr}_{chk}")
            for ct in range(NCT):
                nc.tensor.matmul(
                    pq[:],
                    w_s[:, ct, 128 * gr:128 * (gr + 1)],
                    xt_s[ct][:, 512 * chk:512 * (chk + 1)],
                    start=(ct == 0), stop=(ct == NCT - 1),
                )
                yield
            nc.vector.tensor_scalar_add(
                o_s[:, gr, 512 * chk:512 * (chk + 1)], pq[:],
                b_s[:, gr:gr + 1])
            yield

        def proj_steps(gr, tag):
            # Q, K, then V+transposes interleaved with the K chunks
            for chk in range(NQC):
                yield from emit_proj_group(wq_s, bq_s, qt_s, gr, chk, tag)
            for chk in range(NQC):
                yield from emit_proj_group(wv_s, bv_s, vt_s, gr, chk, tag)
            emit_v_transposes(gr)
            yield
            for chk in range(NQC):
                yield from emit_proj_group(wk_s, bk_s, kt_s, gr, chk, tag)

        def emit_v_transposes(gr):
            # scatter V^T[dh, t] into k-major v_aug via DMA xbar transpose,
            # one 64-row half (one head) per transpose so the output is a
            # contiguous [128, 64] block
            for kt in range(NKT):
                for hh in range(2):
                    nc.sync.dma_start_transpose(
                        v_aug[:, kt, 2 * gr + hh, 0:DH],
                        vt_s[64 * hh:64 * hh + 64, gr,
                             128 * kt:128 * (kt + 1)],
                    )

        if do_proj:
            for _ in proj_steps(0, "s2"):
                pass
        if parts == "proj":
            dump2 = tl.tile([128, 48], F32, tag="dump2")
            nc.vector.tensor_copy(dump2[:, 0:16], qt_s[:, 0, 0:16])
            nc.vector.tensor_copy(dump2[:, 16:32], kt_s[:, 0, 0:16])
            nc.vector.tensor_copy(dump2[:, 32:48], v_aug[:, 0, 0, 0:16])
            nc.sync.dma_start(ot[128:256, 0:48], dump2[:])

        # ---- attention: heads paired per group (concurrent row-strip MMs) ----
        filler = iter(proj_steps(1, "pp")) if do_proj else iter(())
        for gr in range(2 if do_attn else 0):
            hA, hB = 2 * gr, 2 * gr + 1
            ot_A = tl.tile([64, T], F32, tag="otA")
            ot_B = tl.tile([64, T], F32, tag="otB")
            for chk in range(NQC):
                q0 = 512 * chk
                ntA = ps.tile([128, 512], F32, tag="ntA", bufs=1)
                ntB = ps.tile([128, 512], F32, tag="ntB", bufs=1)
                nkt = 4 * chk + 4
                for kt in range(nkt):
                    diag = kt >= 4 * chk
                    w0 = 128 * (kt - 4 * chk) if diag else 0
                    s2 = ps.tile([128, 1024], F32, tag="s2", bufs=2)
                    e2 = wrk.tile([128, 1024], MMDT, tag="e2")
                    ksl = slice(128 * kt, 128 * (kt + 1))
                    qsl = slice(q0 + w0, q0 + 512)
                    nc.tensor.matmul(
                        s2[:, w0:512],
                        kt_s[0:64, gr, ksl], qt_s[0:64, gr, qsl],
                        start=True, stop=True,
                    )
                    nc.tensor.matmul(
                        s2[:, 512 + w0:1024],
                        kt_s[64:128, gr, ksl], qt_s[64:128, gr, qsl],
                        start=True, stop=True,
                    )
                    if diag:
                        nc.vector.tensor_add(
                            s2[:, w0:w0 + 128], s2[:, w0:w0 + 128], mask_s[:])
                        nc.vector.tensor_add(
                            s2[:, 512 + w0:512 + w0 + 128],
                            s2[:, 512 + w0:512 + w0 + 128], mask_s[:])
                    if w0 == 0:
                        nc.scalar.activation(
                            e2[:], s2[:], mybir.ActivationFunctionType.Exp)
                    else:
                        nc.scalar.activation(
                            e2[:, w0:512], s2[:, w0:512],
                            mybir.ActivationFunctionType.Exp)
                        nc.scalar.activation(
                            e2[:, 512 + w0:1024], s2[:, 512 + w0:1024],
                            mybir.ActivationFunctionType.Exp)
                    if do_pv:
                        nc.tensor.matmul(
                            ntA[:, w0:512],
                            v_aug[:, kt, hA, :], e2[:, w0:512],
                            start=(kt == 0), stop=(kt == nkt - 1),
                            skip_group_check=True,
                        )
                        nc.tensor.matmul(
                            ntB[:, w0:512],
                            v_aug[:, kt, hB, :], e2[:, 512 + w0:1024],
                            start=(kt == 0), stop=(kt == nkt - 1),
                            skip_group_check=True,
                        )
                    else:
                        dcp = tl.tile([1, 4], F32, tag="dcp")
                        nc.vector.tensor_copy(dcp[:], e2[0:1, w0:w0 + 4])
                    # opportunistic group-1 projection work between k-tiles
                    for _ in range(2):
                        next(filler, None)
                if not do_pv:
                    continue
                for nt, ot_h in ((ntA, ot_A), (ntB, ot_B)):
                    dr = tl.tile([1, 512], MMDT, tag="dr", bufs=3)
                    nc.vector.tensor_copy(dr[:], nt[DH:DH + 1, :])
                    rbp = ps.tile([64, 512], F32, tag="rb", bufs=1)
                    nc.tensor.matmul(
                        rbp[:], ones_s[0:1, 0:64], dr[:],
                        start=True, stop=True)
                    rb = tl.tile([64, 512], F32, tag="rbs", bufs=3)
                    nc.vector.reciprocal_approx_fast(out=rb[:], in_=rbp[:])
                    nc.vector.tensor_mul(
                        ot_h[:, q0:q0 + 512], nt[0:DH, :], rb[:])
            for _ in filler:
                pass
            if do_pv:
                nc.sync.dma_start(ot[64 * hA:64 * hA + 64, :], ot_A[:])
                nc.sync.dma_start(ot[64 * hB:64 * hB + 64, :], ot_B[:])

    nc.compile()
    return nc


_LOCK = threading.Lock()
_NC = None


def _get_nc():
    global _NC
    with _LOCK:
        if _NC is None:
            _NC = _build()
    return _NC


def _causal_mask_tile():
    kp = np.arange(128)[:, None]
    j = np.arange(128)[None, :]
    return np.where(j >= kp, 0.0, NEG).astype(np.float32)


def _shard_inputs(X, Wq, bq, Wk, bk, Wv, bv):
    X = np.asarray(X, dtype=np.float32)
    Wq = np.asarray(Wq, dtype=np.float32)
    Wk = np.asarray(Wk, dtype=np.float32)
    Wv = np.asarray(Wv, dtype=np.float32)
    bq = np.asarray(bq, dtype=np.float32)
    bk = np.asarray(bk, dtype=np.float32)
    bv = np.asarray(bv, dtype=np.float32)
    s = np.float32(1.0 / np.sqrt(DH))
    mask = _causal_mask_tile()
    ones = np.ones((1, 512), dtype=NPDT)
    in_maps = []
    for core in range(8):
        b, g = divmod(core, 4)
        sl = slice(RPC * g, RPC * (g + 1))
        in_maps.append({
            "xt": np.ascontiguousarray(X[b].T).astype(NPDT),
            "wq": np.ascontiguousarray((Wq[sl] * s).T).astype(NPDT),
            "wk": np.ascontiguousarray(Wk[sl].T).astype(NPDT),
            "wv": np.ascontiguousarray(Wv[sl].T).astype(NPDT),
            "bq": np.ascontiguousarray((bq[sl] * s).reshape(2, 128).T),
            "bk": np.ascontiguousarray(bk[sl].reshape(2, 128).T),
            "bv": np.ascontiguousarray(bv[sl].reshape(2, 128).T),
            "mask": mask,
            "ones": ones,
        })
    return in_maps


def kernel(X, Wq, bq, Wk, bk, Wv, bv):
    nc = _get_nc()
    in_maps = _shard_inputs(X, Wq, bq, Wk, bk, Wv, bv)
    res = run_bass_kernel_spmd(nc, in_maps, core_ids=list(range(8)))
    out = np.empty((B, T, C), dtype=np.float32)
    for core in range(8):
        b, g = divmod(core, 4)
        out[b, :, RPC * g:RPC * (g + 1)] = res.results[core]["ot"].T
    return out



# revision 2
# speedup vs baseline: 1.1070x; 1.1070x over previous
"""Masked multi-head attention on 8 TRN2 NeuronCores.

Sharding: 8 cores = 2 batches x 4 head-groups (4 heads of 64 dims each).
Each core: Q^T/K^T projections (head-dim on partitions) with biases folded
in as K=1 matmuls, V computed directly in key-major layout [keys, vdim]
(xt tiles stationary) into v_aug blocks [64 vd | ones] per head so the PV
matmul yields numerator rows 0-63 and the softmax denominator in row 64 of
one PSUM accumulation. Scores S^T = K^T.T @ Q^T per 128-key tile; the two
heads of a group run as concurrent row-packed K=64 matmuls (tile_position
auto-derived from base partitions). Above-diagonal tiles skipped, diagonal
tiles column-restricted and masked post-exp by a 0/1 triangle multiply.
exp on ScalarE (no max subtraction: scores ~N(0,1)). Scores are emitted
one key-tile ahead so ScalarE (the bottleneck at ~60% of runtime) never
starves; group-1 projections are drained just-in-time into the group-0
attention loop. Output is attn^T bf16; the host transposes/concats/casts.
"""
import threading
from contextlib import ExitStack

import ml_dtypes
import numpy as np

import concourse.bass as bass
import concourse.tile as tile
from concourse import bacc, mybir
from concourse.bass_utils import run_bass_kernel_spmd

F32 = mybir.dt.float32
MMDT = mybir.dt.bfloat16
NPDT = ml_dtypes.bfloat16
EXP = mybir.ActivationFunctionType.Exp

B, T, C = 2, 2048, 1024
H, DH = 16, 64
HPC = 4            # heads per core
RPC = HPC * DH     # 256 output channels per core
NCT = C // 128     # 8 contraction tiles
NQC = T // 512     # 4 query chunks
NKT = T // 128     # 16 key tiles
N_WARM = 20        # PE warmup matmuls during the DMA head


class _Filler:
    """Ordered stream of (tag, generator) emission blocks.

    drain_through(tag) finishes every block up to and including `tag`;
    step(n) advances n yields from the current position (opportunistic
    interleave of projection work into the attention loop).
    """

    def __init__(self, blocks):
        self.blocks = [(t, iter(g)) for t, g in blocks]
        self.i = 0

    def step(self, n=1):
        for _ in range(n):
            while self.i < len(self.blocks):
                try:
                    next(self.blocks[self.i][1])
                    break
                except StopIteration:
                    self.i += 1
            else:
                return

    def drain_through(self, tag):
        idx = None
        for j in range(self.i, len(self.blocks)):
            if self.blocks[j][0] == tag:
                idx = j
                break
        if idx is None:
            return
        while self.i <= idx:
            try:
                next(self.blocks[self.i][1])
            except StopIteration:
                self.i += 1

    def drain_all(self):
        while self.i < len(self.blocks):
            try:
                next(self.blocks[self.i][1])
            except StopIteration:
                self.i += 1


def _build():
    nc = bacc.Bacc("TRN2", target_bir_lowering=False, debug=False)
    xt = nc.dram_tensor("xt", [C, T], MMDT, kind="ExternalInput").ap()
    wq = nc.dram_tensor("wq", [C, RPC], MMDT, kind="ExternalInput").ap()
    wk = nc.dram_tensor("wk", [C, RPC], MMDT, kind="ExternalInput").ap()
    wv = nc.dram_tensor("wv", [C, RPC], MMDT, kind="ExternalInput").ap()
    bq = nc.dram_tensor("bq", [1, RPC], MMDT, kind="ExternalInput").ap()
    bk = nc.dram_tensor("bk", [1, RPC], MMDT, kind="ExternalInput").ap()
    bv = nc.dram_tensor("bv", [1, RPC], MMDT, kind="ExternalInput").ap()
    tri = nc.dram_tensor("tri", [128, 256], MMDT, kind="ExternalInput").ap()
    ones = nc.dram_tensor("ones", [1, 512], MMDT, kind="ExternalInput").ap()
    ot = nc.dram_tensor("ot", [RPC, T], MMDT, kind="ExternalOutput").ap()

    with tile.TileContext(nc) as tc, ExitStack() as ctx:
        per = ctx.enter_context(tc.tile_pool(name="per", bufs=1))
        wrk = ctx.enter_context(tc.tile_pool(name="wrk", bufs=1))
        ps = ctx.enter_context(tc.tile_pool(name="ps", bufs=1, space="PSUM"))

        # ---- persistent SBUF tiles ----
        wq_s = per.tile([128, NCT, RPC], MMDT, tag="wq")
        wk_s = per.tile([128, NCT, RPC], MMDT, tag="wk")
        wv_s = per.tile([128, NCT, RPC], MMDT, tag="wv")
        bq_s = per.tile([1, RPC], MMDT, tag="bq")
        bk_s = per.tile([1, RPC], MMDT, tag="bk")
        bv_s = per.tile([1, RPC], MMDT, tag="bv")
        ones_s = per.tile([1, 512], MMDT, tag="ones")
        tri_s = per.tile([128, 2, 128], MMDT, tag="tri")
        xt_s = per.tile([128, NCT, T], MMDT, tag="xt")
        qt_s = per.tile([128, 2, T], MMDT, tag="qt")
        kt_s = per.tile([128, 2, T], MMDT, tag="kt")
        # per (key tile, group): [hA 64 | one | hB 64 | one]
        v_aug = per.tile([128, NKT, 2, 130], MMDT, tag="vaug")
        junk = per.tile([128, 512], MMDT, tag="junk")

        nc.gpsimd.memset(junk[:], 0.0)
        nc.gpsimd.memset(v_aug[:, :, :, 64:65], 1.0)
        nc.gpsimd.memset(v_aug[:, :, :, 129:130], 1.0)

        # ---- DMA loads (order = availability order) ----
        nc.sync.dma_start(bq_s[:], bq[:])
        nc.sync.dma_start(bk_s[:], bk[:])
        nc.sync.dma_start(bv_s[:], bv[:])
        nc.sync.dma_start(ones_s[:], ones[:])
        nc.sync.dma_start(tri_s[:], tri.rearrange("p (a b) -> p a b", a=2))
        nc.sync.dma_start(wq_s[:], wq.rearrange("(c p) m -> p c m", p=128))
        xt_r = xt.rearrange("(c p) t -> p c t", p=128)
        nc.sync.dma_start(xt_s[:, :, 0:512], xt_r[:, :, 0:512])
        nc.sync.dma_start(wk_s[:], wk.rearrange("(c p) m -> p c m", p=128))
        nc.sync.dma_start(wv_s[:], wv.rearrange("(c p) m -> p c m", p=128))
        for cc in (1, 2, 3):
            nc.sync.dma_start(
                xt_s[:, :, 512 * cc:512 * (cc + 1)],
                xt_r[:, :, 512 * cc:512 * (cc + 1)])

        # warm the exp table while DMAs run
        warm = wrk.tile([1, 8], MMDT, tag="warm")
        nc.scalar.activation(warm[:], junk[0:1, 0:8], EXP)
        # PE warmup stream (junk matmuls, no DMA deps) to lift HAM to 8/8
        for i in range(N_WARM):
            jt = ps.tile([128, 2, 512], F32, tag="s2", bufs=2, name=f"jt{i}")
            nc.tensor.matmul(jt[:, 0, :], junk[:, 0:128], junk[:],
                             start=True, stop=True)

        # ---- projection emitters ----
        def qk_steps(w_s, b_s, o_s, g, c):
            pq = ps.tile([128, 512], F32, tag="pp", bufs=2,
                         name=f"pq_{o_s.tensor.name}_{g}_{c}")
            nc.tensor.matmul(pq[:], b_s[0:1, 128 * g:128 * (g + 1)],
                             ones_s[:], start=True, stop=False)
            yield
            for ct in range(NCT):
                nc.tensor.matmul(
                    pq[:], w_s[:, ct, 128 * g:128 * (g + 1)],
                    xt_s[:, ct, 512 * c:512 * (c + 1)],
                    start=False, stop=(ct == NCT - 1))
                yield
            nc.vector.tensor_copy(o_s[:, g, 512 * c:512 * (c + 1)], pq[:])
            yield

        def v_steps(kt):
            vp = ps.tile([128, 512], F32, tag="pp", bufs=2, name=f"vp{kt}")
            vps = vp[:, 0:RPC]
            nc.tensor.matmul(vps, ones_s[0:1, 0:128], bv_s[:],
                             start=True, stop=False)
            yield
            for ct in range(NCT):
                nc.tensor.matmul(
                    vps, xt_s[:, ct, 128 * kt:128 * (kt + 1)],
                    wv_s[:, ct, :], start=False, stop=(ct == NCT - 1))
                yield
            for g in range(2):
                nc.vector.tensor_copy(
                    v_aug[:, kt, g, 0:64], vp[:, 128 * g:128 * g + 64])
                yield
                nc.vector.tensor_copy(
                    v_aug[:, kt, g, 65:129], vp[:, 128 * g + 64:128 * g + 128])
                yield

        def chain(*gens):
            for gg in gens:
                yield from gg

        def run(gen):
            for _ in gen:
                pass

        # pre-attention: everything attn(g0, c0) needs
        run(qk_steps(wq_s, bq_s, qt_s, 0, 0))
        run(qk_steps(wk_s, bk_s, kt_s, 0, 0))
        for kt in range(4):
            run(v_steps(kt))

        # remaining projections, in need-order, drained JIT + opportunistically
        blocks = []
        for c in (1, 2, 3):
            blocks.append((f"qk0{c}", chain(
                qk_steps(wq_s, bq_s, qt_s, 0, c),
                qk_steps(wk_s, bk_s, kt_s, 0, c))))
            for kt in range(4 * c, 4 * c + 4):
                blocks.append((f"v{kt}", v_steps(kt)))
        for c in range(NQC):
            blocks.append((f"qk1{c}", chain(
                qk_steps(wq_s, bq_s, qt_s, 1, c),
                qk_steps(wk_s, bk_s, kt_s, 1, c))))
        filler = _Filler(blocks)

        # ---- attention ----
        def emit_S(g, c, kt):
            w0 = max(0, 128 * (kt - 4 * c))
            s2_t = ps.tile([128, 2, 512], F32, tag="s2", bufs=2,
                           name=f"s2_{g}_{c}_{kt}")
            ksl = slice(128 * kt, 128 * (kt + 1))
            qsl = slice(512 * c + w0, 512 * (c + 1))
            nc.tensor.matmul(s2_t[:, 0, w0:512], kt_s[0:64, g, ksl],
                             qt_s[0:64, g, qsl], start=True, stop=True)
            nc.tensor.matmul(s2_t[:, 1, w0:512], kt_s[64:128, g, ksl],
                             qt_s[64:128, g, qsl], start=True, stop=True)
            return s2_t, w0

        def make_norm(g, c, ntA, ntB):
            def norm():
                for h, nt in ((0, ntA), (1, ntB)):
                    dr = wrk.tile([1, 512], MMDT, tag="dr", bufs=2,
                                  name=f"dr_{g}_{c}_{h}")
                    nc.vector.tensor_copy(dr[:], nt[64:65, :])
                    rbp = ps.tile([128, 512], F32, tag="pp", bufs=2,
                                  name=f"rbp_{g}_{c}_{h}")
                    nc.tensor.matmul(rbp[0:64, :], ones_s[0:1, 0:64], dr[:],
                                     start=True, stop=True)
                    rb = wrk.tile([64, 512], F32, tag="rb", bufs=2,
                                  name=f"rb_{g}_{c}_{h}")
                    nc.vector.reciprocal_approx_fast(out=rb[:], in_=rbp[0:64, :])
                    oc = wrk.tile([64, 512], MMDT, tag="oc", bufs=4,
                                  name=f"oc_{g}_{c}_{h}")
                    nc.vector.tensor_mul(oc[:], nt[0:64, :], rb[:])
                    nc.sync.dma_start(
                        ot[64 * (2 * g + h):64 * (2 * g + h) + 64,
                           512 * c:512 * (c + 1)], oc[:])
            return norm

        pending_norm = None
        for g in range(2):
            for c in range(NQC):
                if (g, c) != (0, 0):
                    filler.drain_through(f"qk{g}{c}")
                nkt = 4 * c + 4
                ntA = ps.tile([128, 512], F32, tag="ntA", bufs=1,
                              name=f"ntA{g}{c}")
                ntB = ps.tile([128, 512], F32, tag="ntB", bufs=1,
                              name=f"ntB{g}{c}")
                pend = emit_S(g, c, 0)
                if pending_norm is not None:
                    pending_norm()
                    pending_norm = None
                for kt in range(nkt):
                    s2_t, w0 = pend
                    if kt + 1 < nkt:
                        pend = emit_S(g, c, kt + 1)
                    e2 = wrk.tile([128, 2, 512], MMDT, tag="e2", bufs=3,
                                  name=f"e2_{g}_{c}_{kt}")
                    nc.scalar.activation(e2[:, :, w0:512], s2_t[:, :, w0:512],
                                         EXP)
                    if kt >= 4 * c:
                        nc.vector.tensor_mul(
                            e2[:, :, w0:w0 + 128], e2[:, :, w0:w0 + 128],
                            tri_s[:])
                    if g == 0 and kt >= 4:
                        filler.drain_through(f"v{kt}")
                    nc.tensor.matmul(
                        ntA[0:65, w0:512], v_aug[:, kt, g, 0:65],
                        e2[:, 0, w0:512], start=(kt == 0),
                        stop=(kt == nkt - 1), skip_group_check=True)
                    nc.tensor.matmul(
                        ntB[0:65, w0:512], v_aug[:, kt, g, 65:130],
                        e2[:, 1, w0:512], start=(kt == 0),
                        stop=(kt == nkt - 1), skip_group_check=True)
                    filler.step(5)
                pending_norm = make_norm(g, c, ntA, ntB)
        filler.drain_all()
        pending_norm()

    nc.compile()
    return nc


_LOCK = threading.Lock()
_NC = None


def _get_nc():
    global _NC
    with _LOCK:
        if _NC is None:
            _NC = _build()
    return _NC


def _tri_tile():
    p = np.arange(128)[:, None]
    j = np.arange(128)[None, :]
    t = (j >= p).astype(np.float32)
    return np.concatenate([t, t], axis=1).astype(NPDT)


def _shard_inputs(X, Wq, bq, Wk, bk, Wv, bv):
    X = np.asarray(X, dtype=np.float32)
    Wq = np.asarray(Wq, dtype=np.float32)
    Wk = np.asarray(Wk, dtype=np.float32)
    Wv = np.asarray(Wv, dtype=np.float32)
    bq = np.asarray(bq, dtype=np.float32)
    bk = np.asarray(bk, dtype=np.float32)
    bv = np.asarray(bv, dtype=np.float32)
    s = np.float32(1.0 / np.sqrt(DH))
    tri = _tri_tile()
    ones = np.ones((1, 512), dtype=NPDT)
    in_maps = []
    for core in range(8):
        b, gq = divmod(core, 4)
        sl = slice(RPC * gq, RPC * (gq + 1))
        in_maps.append({
            "xt": np.ascontiguousarray(X[b].T).astype(NPDT),
            "wq": np.ascontiguousarray((Wq[sl] * s).T).astype(NPDT),
            "wk": np.ascontiguousarray(Wk[sl].T).astype(NPDT),
            "wv": np.ascontiguousarray(Wv[sl].T).astype(NPDT),
            "bq": (bq[sl] * s).reshape(1, RPC).astype(NPDT),
            "bk": bk[sl].reshape(1, RPC).astype(NPDT),
            "bv": bv[sl].reshape(1, RPC).astype(NPDT),
            "tri": tri,
            "ones": ones,
        })
    return in_maps


def kernel(X, Wq, bq, Wk, bk, Wv, bv):
    nc = _get_nc()
    in_maps = _shard_inputs(X, Wq, bq, Wk, bk, Wv, bv)
    res = run_bass_kernel_spmd(nc, in_maps, core_ids=list(range(8)))
    out = np.empty((B, T, C), dtype=np.float32)
    for core in range(8):
        b, gq = divmod(core, 4)
        out[b, :, RPC * gq:RPC * (gq + 1)] = (
            res.results[core]["ot"].astype(np.float32).T)
    return out
